# revision 73
# baseline (speedup 1.0000x reference)
"""CLUB loss kernel for 8x TRN2 NeuronCores.

Math: the reference computes, per sample b (L=512 positions, D=64 dims):
  mu     = MLP_mu(x);  logvar = tanh(MLP_lv(x));  iv = exp(-logvar)
  positive[d,l] = -(mu - y)^2 * 0.5 * iv
  negative[d,l] = -mean_j (y[d,j] - mu[d,l])^2 * 0.5 * iv
  loss = mean over (b,l) of sum_d (positive - negative)

The pairwise LxL mean collapses via moments of y over positions:
  mean_j (y_j - mu)^2 = Ey2 - 2*mu*Ey + mu^2
so with yd2 = 2*(y - Ey), ysq = y^2, mu = mu_nb + b2:
  loss = -0.5/(B*L) * sum_{b,d,l} [ ((ysq - Ey2) - mu*yd2) * iv ]
       = -0.5/(B*L) * [ sum(ysq*iv) - sum_d Ey2[d]*sum_l iv[d,l]
                        - sum(mu * (yd2*iv)) ]
sum_l iv comes free from exp's accumulator; the three per-dim accumulator
columns are collapsed on-chip by a single ones/ey2-weighted matmul so the
store is one single-packet DMA (a [64,1] store costs 64 tiny descriptors +
16 lazily-arriving semaphore increments, ~5us observed).

Sharding: data-parallel over batch B=8, one sample per core; host does the
tiny final combine.

Precision/speed: fp32 matmul runs at 4 cycles/col on the PE and fp32r (the
1 cycle/col mode) truncates to ~bf16 operand precision anyway — so all
matmul operands (x, W1, W2, relu outputs) are bf16, which also halves the
dominant DMA traffic. y, biases, PSUM accumulation and all elementwise math
stay fp32. Measured loss error vs the fp32 reference: ~2.8e-4 relative.
"""

import sys

if "/opt/trn_rl_repo" not in sys.path:
    sys.path.insert(0, "/opt/trn_rl_repo")

import numpy as np

B, L = 8, 512
XD, YD, H = 192, 64, 128
NCORES = 8
HC = L // 2
WIC = 640  # w1 pack (512) + w2 pack (128)
YBC = 516  # y (512) + b2mu, b2lv, pad, pad

_CACHE: dict = {}


def build_nc(debug: bool = False):
    import concourse.bass as bass
    import concourse.bacc as bacc
    import concourse.tile as tile
    from concourse import mybir

    f32 = mybir.dt.float32
    bf16 = mybir.dt.bfloat16
    AF = mybir.ActivationFunctionType
    OP = mybir.AluOpType

    nc = bacc.Bacc("TRN2", target_bir_lowering=False, debug=debug)

    # one tensor per DMA queue, packed so each queue moves few, large
    # descriptors (descriptor count, not bytes, limits the queues here)
    wi_d = nc.dram_tensor("wi", [128, WIC], bf16, kind="ExternalInput")
    xa_d = nc.dram_tensor("xa", [128, L], bf16, kind="ExternalInput")
    xb_d = nc.dram_tensor("xb", [64, L], bf16, kind="ExternalInput")
    yb_d = nc.dram_tensor("yb", [64, YBC], f32, kind="ExternalInput")
    b1_d = nc.dram_tensor("b1", [128, 2], f32, kind="ExternalInput")
    acc_d = nc.dram_tensor("acc", [4, 2], f32, kind="ExternalOutput")

    with tile.TileContext(nc) as tc:
        with (
            tc.tile_pool(name="sb", bufs=1) as sb,
            tc.tile_pool(name="ps", bufs=1, space=bass.MemorySpace.PSUM) as ps,
            tc.tile_pool(name="hps", bufs=3, space=bass.MemorySpace.PSUM) as hps,
        ):
            ones = sb.tile([64, 1], f32, tag="ones")
            nc.gpsimd.memset(ones, 1.0)

            # input DMAs: w-pack and xa on their own HWDGE rings (they gate
            # the first matmuls), xb first on SWDGE, then y/biases
            wit = sb.tile([128, WIC], bf16, tag="wit")
            nc.scalar.dma_start(out=wit, in_=wi_d[:, :])
            xat = sb.tile([128, L], bf16, tag="xat")
            nc.sync.dma_start(out=xat, in_=xa_d[:, :])
            xbr = sb.tile([128, L], bf16, tag="xbr")
            nc.gpsimd.dma_start(out=xbr[64:128, :], in_=xb_d[:, :])
            ybt = sb.tile([64, YBC], f32, tag="ybt")
            nc.gpsimd.dma_start(out=ybt, in_=yb_d[:, :])
            b1t = sb.tile([128, 2], f32, tag="b1t")
            nc.gpsimd.dma_start(out=b1t, in_=b1_d[:, :])

            w1lvT_a = wit[:, 0:128]
            w1muT_a = wit[:, 128:256]
            w1lvT_b = wit[64:128, 256:384]
            w1muT_b = wit[64:128, 384:512]
            w2lvT = wit[:, 512:576]
            w2muT = wit[:, 576:640]
            b1mu = b1t[:, 0:1]
            b1lv = b1t[:, 1:2]
            xa = xat[:, :]
            xb = xbr[64:128, :]
            y = ybt[:, 0:512]
            b2mu = ybt[:, 512:513]
            b2lv = ybt[:, 513:514]

            # --- moments of y (DVE, early — overlaps the DMA/matmul wait) ---
            sums = sb.tile([64, 2], f32, tag="sums")
            nc.vector.reduce_sum(out=sums[:, 0:1], in_=y, axis=mybir.AxisListType.X)
            ysq = sb.tile([64, L], f32, tag="ysq")
            nc.vector.scalar_tensor_tensor(
                out=ysq, in0=y, scalar=1.0, in1=y,
                op0=OP.mult, op1=OP.mult, accum_out=sums[:, 1:2],
            )
            eyb = sb.tile([64, 2], f32, tag="eyb")
            nc.vector.tensor_scalar_mul(out=eyb, in0=sums, scalar1=1.0 / L)
            ey = eyb[:, 0:1]
            ey2 = eyb[:, 1:2]
            yd2 = sb.tile([64, L], f32, tag="yd2")
            nc.vector.tensor_scalar(
                out=yd2, in0=y, scalar1=ey, scalar2=2.0, op0=OP.subtract, op1=OP.mult
            )

            # --- MLP, both paths chunked over L halves for pipelining.
            # lv half 0 first (its tail is two ACT stages deeper); explicit
            # ordering edges keep the scheduler from starving the lv tail.
            from concourse.tile import add_dep_helper

            acc6 = sb.tile([64, 6], f32, tag="acc6")
            h_lv_s = sb.tile([128, L], bf16, tag="hlvs")
            h_mu_s = sb.tile([128, L], bf16, tag="hmus")
            mm = {}
            act_order = []
            dve_order = []
            for c in range(2):
                cs = slice(c * HC, (c + 1) * HC)
                # layer 1, lv then mu for this half
                h_lv = hps.tile([128, HC], f32, tag="h")
                mm[f"alv{c}"] = nc.tensor.matmul(
                    h_lv, w1lvT_a, xa[:, cs], start=True, stop=False
                )
                mm[f"blv{c}"] = nc.tensor.matmul(
                    h_lv, w1lvT_b, xb[:, cs], start=False, stop=True
                )
                h_mu = hps.tile([128, HC], f32, tag="h")
                mm[f"amu{c}"] = nc.tensor.matmul(
                    h_mu, w1muT_a, xa[:, cs], start=True, stop=False
                )
                mm[f"bmu{c}"] = nc.tensor.matmul(
                    h_mu, w1muT_b, xb[:, cs], start=False, stop=True
                )
                # relu_lv on ACT, relu_mu on DVE
                act_order.append(
                    nc.scalar.activation(
                        out=h_lv_s[:, cs], in_=h_lv, func=AF.Relu, bias=b1lv, scale=1.0
                    )
                )
                dve_order.append(
                    nc.vector.tensor_scalar(
                        out=h_mu_s[:, cs], in0=h_mu, scalar1=b1mu, scalar2=0.0,
                        op0=OP.add, op1=OP.max,
                    )
                )
                # layer 2
                lv_nb = ps.tile([64, HC], f32, tag=f"lvnb{c}")
                mm[f"w2lv{c}"] = nc.tensor.matmul(
                    lv_nb, w2lvT, h_lv_s[:, cs], start=True, stop=True
                )
                mu_nb = ps.tile([64, HC], f32, tag=f"munb{c}")
                mm[f"w2mu{c}"] = nc.tensor.matmul(
                    mu_nb, w2muT, h_mu_s[:, cs], start=True, stop=True
                )
                # lv tail: tanh -> exp (+free sum(iv) via ACT accumulator)
                t1 = sb.tile([64, HC], f32, tag="t1")
                act_order.append(
                    nc.scalar.activation(
                        out=t1, in_=lv_nb, func=AF.Tanh, bias=b2lv, scale=1.0
                    )
                )
                iv = sb.tile([64, HC], f32, tag="iv")
                act_order.append(
                    nc.scalar.activation(out=iv, in_=t1, func=AF.Exp, scale=-1.0)
                )
                dve_order.append(
                    nc.vector.reduce_sum(
                        out=acc6[:, 4 + c : 5 + c], in_=iv,
                        axis=mybir.AxisListType.X,
                    )
                )
                # m2 = (mu_nb + b2mu) * yd2 on DVE
                m2 = sb.tile([64, HC], f32, tag="m2")
                dve_order.append(
                    nc.vector.scalar_tensor_tensor(
                        out=m2, in0=mu_nb, scalar=b2mu, in1=yd2[:, cs],
                        op0=OP.add, op1=OP.mult,
                    )
                )
                # finA1 = sum(ysq * iv), finB = sum(m2 * iv)
                scrA = sb.tile([64, HC], f32, tag="scrA")
                dve_order.append(
                    nc.vector.scalar_tensor_tensor(
                        out=scrA, in0=ysq[:, cs], scalar=1.0, in1=iv,
                        op0=OP.mult, op1=OP.mult, accum_out=acc6[:, 2 * c : 2 * c + 1],
                    )
                )
                scrB = sb.tile([64, HC], f32, tag="scrB")
                dve_order.append(
                    nc.vector.scalar_tensor_tensor(
                        out=scrB, in0=m2, scalar=1.0, in1=iv,
                        op0=OP.mult, op1=OP.mult,
                        accum_out=acc6[:, 2 * c + 1 : 2 * c + 2],
                    )
                )
            # PE stream: half-1 layer-1 matmuls fill the gaps while the relus
            # for half 0 run on ACT/DVE
            pe_order = [
                mm["alv0"], mm["blv0"], mm["amu0"], mm["bmu0"],
                mm["alv1"], mm["w2lv0"], mm["blv1"], mm["w2mu0"],
                mm["amu1"], mm["bmu1"], mm["w2lv1"], mm["w2mu1"],
            ]
            for order in (pe_order, act_order, dve_order):
                for a, b in zip(order[1:], order[:-1]):
                    add_dep_helper(a.ins, b.ins, sync=False, reason="stream-order")
            # two-matmul collapse: the ey2 dot-product over the iv-sums is
            # ready ~1us before the last finB, so it runs early and off the
            # critical path; its PSUM tile reuses a freed h-pool slot.
            civ_ps = hps.tile([2, 1], f32, tag="h")
            mm_civ = nc.tensor.matmul(civ_ps, acc6[:, 4:6], ey2, start=True, stop=True)
            acc_ps = ps.tile([4, 1], f32, tag="accps")
            mm_acc = nc.tensor.matmul(acc_ps, acc6[:, 0:4], ones, start=True, stop=True)
            add_dep_helper(mm_acc.ins, mm_civ.ins, sync=False, reason="civ-first")
            acc_sb = sb.tile([4, 2], f32, tag="accsb")
            nc.vector.tensor_copy(acc_sb[0:2, 1:2], civ_ps)
            nc.vector.tensor_copy(acc_sb[:, 0:1], acc_ps)
            nc.sync.dma_start(out=acc_d[:, :], in_=acc_sb, single_packet=True)

    nc.compile()
    return nc


def pack_inputs(inputs: dict) -> list[dict]:
    import ml_dtypes

    bf = ml_dtypes.bfloat16
    x = np.asarray(inputs["x_samples"], dtype=np.float32)
    y = np.ascontiguousarray(np.asarray(inputs["y_samples"], dtype=np.float32))
    mu_W1 = np.asarray(inputs["mu_W1"], dtype=np.float32)
    mu_b1 = np.asarray(inputs["mu_b1"], dtype=np.float32)
    mu_W2 = np.asarray(inputs["mu_W2"], dtype=np.float32)
    mu_b2 = np.asarray(inputs["mu_b2"], dtype=np.float32)
    lv_W1 = np.asarray(inputs["lv_W1"], dtype=np.float32)
    lv_b1 = np.asarray(inputs["lv_b1"], dtype=np.float32)
    lv_W2 = np.asarray(inputs["lv_W2"], dtype=np.float32)
    lv_b2 = np.asarray(inputs["lv_b2"], dtype=np.float32)

    wi = np.zeros((128, WIC), bf)
    w1muT = mu_W1.T  # [192, 128]
    w1lvT = lv_W1.T
    wi[:, 0:128] = w1lvT[0:128].astype(bf)
    wi[:, 128:256] = w1muT[0:128].astype(bf)
    wi[64:128, 256:384] = w1lvT[128:192].astype(bf)
    wi[64:128, 384:512] = w1muT[128:192].astype(bf)
    wi[:, 512:576] = lv_W2.T.astype(bf)
    wi[:, 576:640] = mu_W2.T.astype(bf)
    b1 = np.ascontiguousarray(np.stack([mu_b1, lv_b1], axis=1))  # [128, 2]

    xb16 = x.astype(bf)
    in_maps = []
    for b in range(NCORES):
        yb = np.zeros((64, YBC), np.float32)
        yb[:, 0:512] = y[b]
        yb[:, 512] = mu_b2
        yb[:, 513] = lv_b2
        in_maps.append(
            {
                "wi": wi,
                "xa": np.ascontiguousarray(xb16[b, 0:128]),
                "xb": np.ascontiguousarray(xb16[b, 128:192]),
                "yb": yb,
                "b1": b1,
            }
        )
    return in_maps


def _combine(results) -> float:
    tot = 0.0
    for r in results:
        a = r["acc"].astype(np.float64)  # [4, 2]
        # col 0: finA1_0, finB_0, finA1_1, finB_1; col 1 rows 0:2: ey2-dot of siv halves
        tot += (a[0, 0] + a[2, 0]) - (a[1, 0] + a[3, 0]) - (a[0, 1] + a[1, 1])
    return tot


def kernel(**inputs) -> np.ndarray:
    from concourse.bass_utils import run_bass_kernel_spmd

    if "nc" not in _CACHE:
        _CACHE["nc"] = build_nc(debug=False)
    nc = _CACHE["nc"]

    in_maps = pack_inputs(inputs)
    res = run_bass_kernel_spmd(nc, in_maps, core_ids=list(range(NCORES)))
    loss = -0.5 * _combine(res.results) / (B * L)
    return np.array(loss, dtype=np.float32)


# revision 75
# speedup vs baseline: 1.0693x; 1.0693x over previous
"""CLUB loss kernel for 8x TRN2 NeuronCores.

Math: the reference computes, per sample b (L=512 positions, D=64 dims):
  mu     = MLP_mu(x);  logvar = tanh(MLP_lv(x));  iv = exp(-logvar)
  positive[d,l] = -(mu - y)^2 * 0.5 * iv
  negative[d,l] = -mean_j (y[d,j] - mu[d,l])^2 * 0.5 * iv
  loss = mean over (b,l) of sum_d (positive - negative)

The pairwise LxL mean collapses via moments of y over positions:
  mean_j (y_j - mu)^2 = Ey2 - 2*mu*Ey + mu^2
so with yd2 = 2*(y - Ey), ysq = y^2, mu = mu_nb + b2:
  loss = -0.5/(B*L) * sum_{b,d,l} [ ((ysq - Ey2) - mu*yd2) * iv ]
       = -0.5/(B*L) * [ sum(ysq*iv) - sum_d Ey2[d]*sum_l iv[d,l]
                        - sum(mu * (yd2*iv)) ]
sum_l iv comes free from exp's accumulator; the three per-dim accumulator
columns are collapsed on-chip by a single ones/ey2-weighted matmul so the
store is one single-packet DMA (a [64,1] store costs 64 tiny descriptors +
16 lazily-arriving semaphore increments, ~5us observed).

Sharding: data-parallel over batch B=8, one sample per core; host does the
tiny final combine.

Precision/speed: fp32 matmul runs at 4 cycles/col on the PE and fp32r (the
1 cycle/col mode) truncates to ~bf16 operand precision anyway — so all
matmul operands (x, W1, W2, relu outputs) are bf16, which also halves the
dominant DMA traffic. y, biases, PSUM accumulation and all elementwise math
stay fp32. Measured loss error vs the fp32 reference: ~2.8e-4 relative.
"""

import sys

if "/opt/trn_rl_repo" not in sys.path:
    sys.path.insert(0, "/opt/trn_rl_repo")

import numpy as np

B, L = 8, 512
XD, YD, H = 192, 64, 128
NCORES = 8
HC = L // 2
WIC = 640  # w1 pack (512) + w2 pack (128)
YBC = 516  # y (512) + b2mu, b2lv, pad, pad

_CACHE: dict = {}


def build_nc(debug: bool = False):
    import concourse.bass as bass
    import concourse.bacc as bacc
    import concourse.tile as tile
    from concourse import mybir

    f32 = mybir.dt.float32
    bf16 = mybir.dt.bfloat16
    AF = mybir.ActivationFunctionType
    OP = mybir.AluOpType

    nc = bacc.Bacc("TRN2", target_bir_lowering=False, debug=debug)

    # one tensor per DMA queue, packed so each queue moves few, large
    # descriptors (descriptor count, not bytes, limits the queues here)
    wi_d = nc.dram_tensor("wi", [128, WIC], bf16, kind="ExternalInput")
    xa_d = nc.dram_tensor("xa", [128, L], bf16, kind="ExternalInput")
    xb_d = nc.dram_tensor("xb", [64, L], bf16, kind="ExternalInput")
    yb_d = nc.dram_tensor("yb", [64, YBC], f32, kind="ExternalInput")
    b1_d = nc.dram_tensor("b1", [128, 2], f32, kind="ExternalInput")
    acc_d = nc.dram_tensor("acc", [4, 2], f32, kind="ExternalOutput")

    with tile.TileContext(nc) as tc:
        with (
            tc.tile_pool(name="sb", bufs=1) as sb,
            tc.tile_pool(name="ps", bufs=1, space=bass.MemorySpace.PSUM) as ps,
            tc.tile_pool(name="hps", bufs=3, space=bass.MemorySpace.PSUM) as hps,
        ):
            ones = sb.tile([64, 1], f32, tag="ones")
            nc.gpsimd.memset(ones, 1.0)

            # input DMAs: w-pack and xa on their own HWDGE rings (they gate
            # the first matmuls), xb first on SWDGE, then y/biases
            wit = sb.tile([128, WIC], bf16, tag="wit")
            nc.scalar.dma_start(out=wit, in_=wi_d[:, :])
            xat = sb.tile([128, L], bf16, tag="xat")
            nc.sync.dma_start(out=xat, in_=xa_d[:, :])
            xbr = sb.tile([128, L], bf16, tag="xbr")
            nc.gpsimd.dma_start(out=xbr[64:128, :], in_=xb_d[:, :])
            ybt = sb.tile([64, YBC], f32, tag="ybt")
            nc.gpsimd.dma_start(out=ybt, in_=yb_d[:, :])
            b1t = sb.tile([128, 2], f32, tag="b1t")
            nc.gpsimd.dma_start(out=b1t, in_=b1_d[:, :])

            w1lvT_a = wit[:, 0:128]
            w1muT_a = wit[:, 128:256]
            w1lvT_b = wit[64:128, 256:384]
            w1muT_b = wit[64:128, 384:512]
            w2lvT = wit[:, 512:576]
            w2muT = wit[:, 576:640]
            b1mu = b1t[:, 0:1]
            b1lv = b1t[:, 1:2]
            xa = xat[:, :]
            xb = xbr[64:128, :]
            y = ybt[:, 0:512]
            b2mu = ybt[:, 512:513]
            b2lv = ybt[:, 513:514]

            # --- moments of y (DVE, early — overlaps the DMA/matmul wait) ---
            sums = sb.tile([64, 2], f32, tag="sums")
            nc.vector.reduce_sum(out=sums[:, 0:1], in_=y, axis=mybir.AxisListType.X)
            ysq = sb.tile([64, L], f32, tag="ysq")
            nc.vector.scalar_tensor_tensor(
                out=ysq, in0=y, scalar=1.0, in1=y,
                op0=OP.mult, op1=OP.mult, accum_out=sums[:, 1:2],
            )
            eyb = sb.tile([64, 2], f32, tag="eyb")
            nc.vector.tensor_scalar_mul(out=eyb, in0=sums, scalar1=1.0 / L)
            ey = eyb[:, 0:1]
            ey2 = eyb[:, 1:2]
            yd2 = sb.tile([64, L], f32, tag="yd2")
            nc.vector.tensor_scalar(
                out=yd2, in0=y, scalar1=ey, scalar2=2.0, op0=OP.subtract, op1=OP.mult
            )

            # --- MLP, both paths chunked over L halves for pipelining.
            # lv half 0 first (its tail is two ACT stages deeper); explicit
            # ordering edges keep the scheduler from starving the lv tail.
            from concourse.tile import add_dep_helper

            acc6 = sb.tile([64, 6], f32, tag="acc6")
            h_lv_s = sb.tile([128, L], bf16, tag="hlvs")
            h_mu_s = sb.tile([128, L], bf16, tag="hmus")
            mm = {}
            act_order = []
            dve_order = []
            for c in range(2):
                cs = slice(c * HC, (c + 1) * HC)
                # layer 1, lv then mu for this half
                h_lv = hps.tile([128, HC], f32, tag="h")
                mm[f"alv{c}"] = nc.tensor.matmul(
                    h_lv, w1lvT_a, xa[:, cs], start=True, stop=False
                )
                mm[f"blv{c}"] = nc.tensor.matmul(
                    h_lv, w1lvT_b, xb[:, cs], start=False, stop=True
                )
                h_mu = hps.tile([128, HC], f32, tag="h")
                mm[f"amu{c}"] = nc.tensor.matmul(
                    h_mu, w1muT_a, xa[:, cs], start=True, stop=False
                )
                mm[f"bmu{c}"] = nc.tensor.matmul(
                    h_mu, w1muT_b, xb[:, cs], start=False, stop=True
                )
                # relu_lv on ACT, relu_mu on DVE
                act_order.append(
                    nc.scalar.activation(
                        out=h_lv_s[:, cs], in_=h_lv, func=AF.Relu, bias=b1lv, scale=1.0
                    )
                )
                dve_order.append(
                    nc.vector.tensor_scalar(
                        out=h_mu_s[:, cs], in0=h_mu, scalar1=b1mu, scalar2=0.0,
                        op0=OP.add, op1=OP.max,
                    )
                )
                # layer 2
                lv_nb = ps.tile([64, HC], f32, tag=f"lvnb{c}")
                mm[f"w2lv{c}"] = nc.tensor.matmul(
                    lv_nb, w2lvT, h_lv_s[:, cs], start=True, stop=True
                )
                mu_nb = ps.tile([64, HC], f32, tag=f"munb{c}")
                mm[f"w2mu{c}"] = nc.tensor.matmul(
                    mu_nb, w2muT, h_mu_s[:, cs], start=True, stop=True
                )
                # lv tail: tanh -> exp (+free sum(iv) via ACT accumulator)
                t1 = sb.tile([64, HC], f32, tag="t1")
                act_order.append(
                    nc.scalar.activation(
                        out=t1, in_=lv_nb, func=AF.Tanh, bias=b2lv, scale=1.0
                    )
                )
                iv = sb.tile([64, HC], f32, tag="iv")
                act_order.append(
                    nc.scalar.activation(out=iv, in_=t1, func=AF.Exp, scale=-1.0)
                )
                dve_order.append(
                    nc.vector.reduce_sum(
                        out=acc6[:, 4 + c : 5 + c], in_=iv,
                        axis=mybir.AxisListType.X,
                    )
                )
                # m2 = (mu_nb + b2mu) * yd2 on DVE
                m2 = sb.tile([64, HC], f32, tag="m2")
                dve_order.append(
                    nc.vector.scalar_tensor_tensor(
                        out=m2, in0=mu_nb, scalar=b2mu, in1=yd2[:, cs],
                        op0=OP.add, op1=OP.mult,
                    )
                )
                # finA1 = sum(ysq * iv), finB = sum(m2 * iv)
                scrA = sb.tile([64, HC], f32, tag="scrA")
                dve_order.append(
                    nc.vector.scalar_tensor_tensor(
                        out=scrA, in0=ysq[:, cs], scalar=1.0, in1=iv,
                        op0=OP.mult, op1=OP.mult, accum_out=acc6[:, 2 * c : 2 * c + 1],
                    )
                )
                scrB = sb.tile([64, HC], f32, tag="scrB")
                dve_order.append(
                    nc.vector.scalar_tensor_tensor(
                        out=scrB, in0=m2, scalar=1.0, in1=iv,
                        op0=OP.mult, op1=OP.mult,
                        accum_out=acc6[:, 2 * c + 1 : 2 * c + 2],
                    )
                )
            # PE stream: half-1 layer-1 matmuls fill the gaps while the relus
            # for half 0 run on ACT/DVE
            pe_order = [
                mm["alv0"], mm["blv0"], mm["amu0"], mm["bmu0"],
                mm["alv1"], mm["w2lv0"], mm["blv1"], mm["w2mu0"],
                mm["amu1"], mm["bmu1"], mm["w2lv1"], mm["w2mu1"],
            ]
            for order in (pe_order, act_order, dve_order):
                for a, b in zip(order[1:], order[:-1]):
                    add_dep_helper(a.ins, b.ins, sync=False, reason="stream-order")
            # two-matmul collapse: the ey2 dot-product over the iv-sums is
            # ready ~1us before the last finB, so it runs early and off the
            # critical path; its PSUM tile reuses a freed h-pool slot.
            civ_ps = hps.tile([2, 1], f32, tag="h")
            mm_civ = nc.tensor.matmul(civ_ps, acc6[:, 4:6], ey2, start=True, stop=True)
            acc_ps = ps.tile([4, 1], f32, tag="accps")
            mm_acc = nc.tensor.matmul(acc_ps, acc6[:, 0:4], ones, start=True, stop=True)
            add_dep_helper(mm_acc.ins, mm_civ.ins, sync=False, reason="civ-first")
            acc_sb = sb.tile([4, 2], f32, tag="accsb")
            nc.vector.tensor_copy(acc_sb[0:2, 1:2], civ_ps)
            nc.vector.tensor_copy(acc_sb[:, 0:1], acc_ps)
            nc.sync.dma_start(out=acc_d[:, :], in_=acc_sb, single_packet=True)

    nc.compile()
    return nc


def pack_inputs(inputs: dict) -> list[dict]:
    import ml_dtypes

    bf = ml_dtypes.bfloat16
    x = np.asarray(inputs["x_samples"], dtype=np.float32)
    y = np.ascontiguousarray(np.asarray(inputs["y_samples"], dtype=np.float32))
    mu_W1 = np.asarray(inputs["mu_W1"], dtype=np.float32)
    mu_b1 = np.asarray(inputs["mu_b1"], dtype=np.float32)
    mu_W2 = np.asarray(inputs["mu_W2"], dtype=np.float32)
    mu_b2 = np.asarray(inputs["mu_b2"], dtype=np.float32)
    lv_W1 = np.asarray(inputs["lv_W1"], dtype=np.float32)
    lv_b1 = np.asarray(inputs["lv_b1"], dtype=np.float32)
    lv_W2 = np.asarray(inputs["lv_W2"], dtype=np.float32)
    lv_b2 = np.asarray(inputs["lv_b2"], dtype=np.float32)

    wi = np.zeros((128, WIC), bf)
    w1muT = mu_W1.T  # [192, 128]
    w1lvT = lv_W1.T
    wi[:, 0:128] = w1lvT[0:128].astype(bf)
    wi[:, 128:256] = w1muT[0:128].astype(bf)
    wi[64:128, 256:384] = w1lvT[128:192].astype(bf)
    wi[64:128, 384:512] = w1muT[128:192].astype(bf)
    wi[:, 512:576] = lv_W2.T.astype(bf)
    wi[:, 576:640] = mu_W2.T.astype(bf)
    b1 = np.ascontiguousarray(np.stack([mu_b1, lv_b1], axis=1))  # [128, 2]

    xb16 = x.astype(bf)
    in_maps = []
    for b in range(NCORES):
        yb = np.zeros((64, YBC), np.float32)
        yb[:, 0:512] = y[b]
        yb[:, 512] = mu_b2
        yb[:, 513] = lv_b2
        in_maps.append(
            {
                "wi": wi,
                "xa": np.ascontiguousarray(xb16[b, 0:128]),
                "xb": np.ascontiguousarray(xb16[b, 128:192]),
                "yb": yb,
                "b1": b1,
            }
        )
    return in_maps


def _combine(results) -> float:
    tot = 0.0
    for r in results:
        a = r["acc"].astype(np.float64)  # [4, 2]
        # col 0: finA1_0, finB_0, finA1_1, finB_1; col 1 rows 0:2: ey2-dot of siv halves
        tot += (a[0, 0] + a[2, 0]) - (a[1, 0] + a[3, 0]) - (a[0, 1] + a[1, 1])
    return tot


def kernel(**inputs) -> np.ndarray:
    from concourse.bass_utils import run_bass_kernel_spmd

    if "nc" not in _CACHE:
        _CACHE["nc"] = build_nc(debug=False)
    nc = _CACHE["nc"]

    in_maps = pack_inputs(inputs)
    res = run_bass_kernel_spmd(nc, in_maps, core_ids=list(range(NCORES)))
    loss = -0.5 * _combine(res.results) / (B * L)
    return np.array(loss, dtype=np.float32)
